# revision 10
# baseline (speedup 1.0000x reference)
"""BitNet DiT on 8 Trainium2 NeuronCores — data-parallel over batch (2 images/core).

Host: patchify, time-embedding + adaLN modulation vectors, BitNet weight
quantization (ternary * per-tensor scale) -> bf16 upload.
Device: full 12-block DiT forward per core in a single Bass/Tile kernel.
BitNet matmuls run as exact integer arithmetic in bf16 (|values| <= 127,
fp32 accumulate). Attention runs in fp32 via transposed-logits + ones-column
softmax-denominator trick.
"""
import math
import os
import sys
import numpy as np

sys.path.insert(0, "/opt/trn_rl_repo")

import ml_dtypes  # noqa: E402
import concourse.bass as bass  # noqa: E402
import concourse.mybir as mybir  # noqa: E402
import concourse.tile as tile  # noqa: E402
from concourse import bacc  # noqa: E402
from concourse.bass_utils import run_bass_kernel_spmd  # noqa: E402
from concourse.masks import make_identity  # noqa: E402

F32 = mybir.dt.float32
BF16 = mybir.dt.bfloat16
AX = mybir.AxisListType
OP = mybir.AluOpType
AF = mybir.ActivationFunctionType

DIM = 768
DEPTH = int(os.environ.get("KERNEL_DEPTH", "12"))
HEADS = 12
HD = 64
PATCH = 16
IMG = 256
CIN = 3
HID = 4 * DIM
EPS = 1e-6
P = 128
T = 512            # tokens per core (2 images x 256)
NT = T // P        # 4 token tiles
NTOK = 256         # tokens per image
KD = DIM // P      # 6
KH = HID // P      # 24
MAGIC = float(np.float32(3 * 2**22))  # 12582912.0 RNE round-to-int magic
INV127 = 1.0 / 127.0

_CACHED = {}


def _mm_chunks(n):
    """Split output width n into <=512 chunks."""
    out = []
    s = 0
    while s < n:
        e = min(s + 512, n)
        out.append((s, e))
        s = e
    return out


def build_program(depth=DEPTH):
    nc = bacc.Bacc("TRN2", target_bir_lowering=False, debug=False, num_devices=8)

    xpT_d = nc.declare_dram_parameter("xpT", [DIM, T], F32, isOutput=False)
    posb_d = nc.declare_dram_parameter("posb", [NTOK, DIM], F32, isOutput=False)
    patchWT_d = nc.declare_dram_parameter("patchWT", [DIM, DIM], F32, isOutput=False)
    headWT_d = nc.declare_dram_parameter("headWT", [DIM, DIM], F32, isOutput=False)
    headb_d = nc.declare_dram_parameter("headb", [1, DIM], F32, isOutput=False)
    wqkv_d = nc.declare_dram_parameter("wqkv", [depth, DIM, 3 * DIM], BF16, isOutput=False)
    wproj_d = nc.declare_dram_parameter("wproj", [depth, DIM, DIM], BF16, isOutput=False)
    wfc1_d = nc.declare_dram_parameter("wfc1", [depth, DIM, HID], BF16, isOutput=False)
    wfc2_d = nc.declare_dram_parameter("wfc2", [depth, HID, DIM], BF16, isOutput=False)
    # modulation vectors: [block, norm(2), img(2), A/B(2), 768]
    mods_d = nc.declare_dram_parameter("mods", [depth, 2, P, 2, 2, DIM], F32, isOutput=False)
    wscl_d = nc.declare_dram_parameter("wscl", [1, 4 * depth], F32, isOutput=False)
    out_d = nc.declare_dram_parameter("zout", [T, DIM], F32, isOutput=True)

    with tile.TileContext(nc) as tc:
        from contextlib import ExitStack
        with ExitStack() as _ctx:
            constp = _ctx.enter_context(tc.tile_pool(name="const", bufs=1))
            residp = _ctx.enter_context(tc.tile_pool(name="resid", bufs=1))
            fm6p = _ctx.enter_context(tc.tile_pool(name="fm6", bufs=2))
            xqTp = _ctx.enter_context(tc.tile_pool(name="xqT", bufs=1))
            wp = _ctx.enter_context(tc.tile_pool(name="w", bufs=2))
            modp = _ctx.enter_context(tc.tile_pool(name="mod", bufs=2))
            tmp_ = _ctx.enter_context(tc.tile_pool(name="tm", bufs=3))
            gp = _ctx.enter_context(tc.tile_pool(name="g", bufs=2))
            xqp = _ctx.enter_context(tc.tile_pool(name="xq", bufs=1))
            eTp = _ctx.enter_context(tc.tile_pool(name="eT", bufs=2))
            scp = _ctx.enter_context(tc.tile_pool(name="sc", bufs=32))
            rowp = _ctx.enter_context(tc.tile_pool(name="row", bufs=2))
            ps_mm = _ctx.enter_context(tc.tile_pool(name="ps_mm", bufs=2, space="PSUM"))
            ps_tp = _ctx.enter_context(tc.tile_pool(name="ps_tp", bufs=2, space="PSUM"))
            ps_lt = _ctx.enter_context(tc.tile_pool(name="ps_lt", bufs=2, space="PSUM"))
            ps_oa = _ctx.enter_context(tc.tile_pool(name="ps_oa", bufs=2, space="PSUM"))

            idf = constp.tile([P, P], F32)
            make_identity(nc, idf[:])
            idb = constp.tile([P, P], BF16)
            nc.vector.tensor_copy(idb[:], idf[:])

            # broadcast w_scales/127 to all partitions
            wsrow = constp.tile([1, 4 * depth], F32)
            nc.sync.dma_start(wsrow[:], wscl_d[:])
            wsb = constp.tile([P, 4 * depth], F32)
            nc.gpsimd.partition_broadcast(wsb[:], wsrow[0:1, :])

            z = residp.tile([P, NT, DIM], F32)
            v_aug = residp.tile([P, NT, HEADS, HD + 1], F32)
            nc.vector.memset(v_aug[:, :, :, HD], 1.0)
            o_tm = residp.tile([P, NT, DIM], F32)



            # ---------------- patch embed ----------------
            posb_sb = wp.tile([P, 2, DIM], F32, tag="w")
            nc.sync.dma_start(posb_sb[:], posb_d.rearrange("(a p) d -> p a d", p=P))
            xpT = fm6p.tile([P, KD, T], F32, tag="fm6")
            nc.sync.dma_start(xpT[:], xpT_d.rearrange("(o p) t -> p o t", p=P))
            pw = wp.tile([P, KD, DIM], F32, tag="w")
            nc.sync.dma_start(pw[:], patchWT_d.rearrange("(o p) d -> p o d", p=P))
            for t in range(NT):
                for (cs, ce) in _mm_chunks(DIM):
                    pt = ps_mm.tile([P, 512], F32, tag="mm", name="pmm")[:, : ce - cs]
                    for k in range(KD):
                        nc.tensor.matmul(pt[:], xpT[:, k, t * P:(t + 1) * P],
                                         pw[:, k, cs:ce], start=(k == 0), stop=(k == KD - 1))
                    nc.vector.tensor_tensor(z[:, t, cs:ce], pt[:], posb_sb[:, t % 2, cs:ce], OP.add)

            def load_w(dram, b, kchunks, width, dtype=BF16):
                """Stage one linear's transposed weights as two tiles."""
                half = kchunks // 2
                tiles = []
                for i in range(2):
                    wt = wp.tile([P, half, width], dtype, tag="w")
                    nc.sync.dma_start(
                        wt[:],
                        dram[b, i * half * P:(i + 1) * half * P, :].rearrange(
                            "(o p) f -> p o f", p=P))
                    tiles.append(wt)
                return tiles, half

            def quantize(src_ap, t, ws_idx, kchunks, xqT_tile):
                """Per-token quantize src [128, kchunks*128] -> bf16 ints in
                xqT_tile[:, k, t*128:+128]; returns c = clip(amax)*ws/127 [128,1]."""
                width = kchunks * P
                amax = scp.tile([P, 1], F32, tag="sc")
                nc.vector.tensor_reduce(amax[:], src_ap, axis=AX.X, op=OP.max,
                                        apply_absolute_value=True)
                amax_c = scp.tile([P, 1], F32, tag="sc")
                nc.vector.tensor_scalar_max(amax_c[:], amax[:], 1e-5)
                r = scp.tile([P, 1], F32, tag="sc")
                nc.vector.reciprocal(r[:], amax_c[:])
                s127 = scp.tile([P, 1], F32, tag="sc")
                nc.vector.tensor_scalar_mul(s127[:], r[:], 127.0)
                c = scp.tile([P, 1], F32, tag="sc")
                nc.vector.tensor_scalar(c[:], amax_c[:], wsb[:, ws_idx:ws_idx + 1], None,
                                        OP.mult)
                tmp = gp.tile([P, HID], F32, tag="g", name="qtmp")[:, :width]
                nc.vector.tensor_scalar(tmp[:], src_ap, s127[:], MAGIC, OP.mult, OP.add)
                xq = xqp.tile([P, HID], BF16, tag="xq", name="xq")[:, :width]
                nc.vector.tensor_scalar(xq[:], tmp[:], MAGIC, None, OP.subtract)
                for k in range(kchunks):
                    ptb = ps_tp.tile([P, P], BF16, tag="tp")
                    nc.tensor.transpose(ptb[:], xq[:, k * P:(k + 1) * P], idb[:])
                    nc.vector.tensor_copy(xqT_tile[:, k, t * P:(t + 1) * P], ptb[:])
                return c

            def rstd_batch(ntiles=NT):
                """rmsnorm stats for all token tiles, ACT funcs batched."""
                sq = tmp_.tile([P, DIM], F32, tag="tm", name="sqscratch")
                ssqs = [scp.tile([P, 1], F32, tag="sc", name="ssq") for _ in range(ntiles)]
                for t in range(ntiles):
                    nc.scalar.activation(sq[:], z[:, t, :], AF.Square, accum_out=ssqs[t][:])
                mss = [scp.tile([P, 1], F32, tag="sc", name="ms") for _ in range(ntiles)]
                for t in range(ntiles):
                    nc.vector.tensor_scalar(mss[t][:], ssqs[t][:], 1.0 / DIM, EPS, OP.mult, OP.add)
                lns = [scp.tile([P, 1], F32, tag="sc", name="lnv") for _ in range(ntiles)]
                for t in range(ntiles):
                    nc.scalar.activation(lns[t][:], mss[t][:], AF.Ln)
                rstds = [scp.tile([P, 1], F32, tag="sc", name="rstd") for _ in range(ntiles)]
                for t in range(ntiles):
                    nc.scalar.activation(rstds[t][:], lns[t][:], AF.Exp, scale=-0.5)
                return rstds

            def norm_mod(t, mt, rstd):
                img = t // 2
                hn = tmp_.tile([P, DIM], F32, tag="tm")
                nc.vector.tensor_scalar_mul(hn[:], z[:, t, :], rstd[:])
                h = tmp_.tile([P, DIM], F32, tag="tm")
                nc.vector.tensor_tensor(h[:], hn[:], mt[:, img, 0, :], OP.mult)
                nc.vector.tensor_tensor(h[:], h[:], mt[:, img, 1, :], OP.add)
                return h

            xqT = xqTp.tile([P, KH, T], BF16, tag="xqT")

            for b in range(depth):
                # --- modulation vectors for this block (both norms) ---
                import contextlib
                def sc_(nm):
                    return nc.named_scope(f"b{b}_{nm}") if b == 5 else contextlib.nullcontext()
                mt1 = modp.tile([P, 2, 2, DIM], F32, tag="mod")
                nc.sync.dma_start(mt1[:], mods_d[b, 0])
                mt2 = modp.tile([P, 2, 2, DIM], F32, tag="mod")
                nc.sync.dma_start(mt2[:], mods_d[b, 1])

                wq_tiles, wq_half = load_w(wqkv_d, b, KD, 3 * DIM)

                # --- phase 1: norm1 + modulate + quantize -> xqT ---
                cq8s, c_list = [], []
                with sc_("p1norm"):
                    rstds1 = rstd_batch()
                    for t in range(NT):
                        h = norm_mod(t, mt1, rstds1[t])
                        c = quantize(h[:], t, 4 * b + 0, KD, xqT)
                        cq8 = scp.tile([P, 1], F32, tag="sc")
                        nc.vector.tensor_scalar_mul(cq8[:], c[:], 0.125)
                        cq8s.append(cq8)
                        c_list.append(c)

                # --- phase 2: qkv matmul + evac ---
                q_fm = fm6p.tile([P, KD, T], mybir.dt.float32r, tag="fm6")
                k_fm = fm6p.tile([P, KD, T], mybir.dt.float32r, tag="fm6")
                _p2 = _ctx2 = None
                _p2 = sc_("p2qkv"); _p2.__enter__()
                for t in range(NT):
                    q_tm = tmp_.tile([P, DIM], F32, tag="tm")
                    k_tm = tmp_.tile([P, DIM], F32, tag="tm")
                    for (cs, ce) in _mm_chunks(3 * DIM):
                        pt = ps_mm.tile([P, 512], F32, tag="mm", name="pmm")[:, : ce - cs]
                        for k in range(KD):
                            wt = wq_tiles[k // wq_half]
                            nc.tensor.matmul(pt[:], xqT[:, k, t * P:(t + 1) * P],
                                             wt[:, k % wq_half, cs:ce],
                                             start=(k == 0), stop=(k == KD - 1))
                        # evac by q/k/v boundary
                        segs = []
                        if cs < DIM:
                            segs.append((cs, min(ce, DIM), "q"))
                        if ce > DIM and cs < 2 * DIM:
                            segs.append((max(cs, DIM), min(ce, 2 * DIM), "k"))
                        if ce > 2 * DIM:
                            segs.append((max(cs, 2 * DIM), ce, "v"))
                        for (s0, s1, kind) in segs:
                            po = pt[:, s0 - cs:s1 - cs]
                            if kind == "q":
                                nc.scalar.activation(q_tm[:, s0:s1], po, AF.Identity,
                                                     scale=cq8s[t][:])
                            elif kind == "k":
                                nc.scalar.activation(k_tm[:, s0 - DIM:s1 - DIM], po,
                                                     AF.Identity, scale=c_list[t][:])
                            else:
                                h0 = (s0 - 2 * DIM) // HD
                                h1 = (s1 - 2 * DIM) // HD
                                nc.scalar.activation(
                                    v_aug[:, t, h0:h1, 0:HD], po, AF.Identity,
                                    scale=c_list[t][:])
                    # transpose q,k to feature-major
                    for k in range(KD):
                        ptq = ps_tp.tile([P, P], F32, tag="tp")
                        nc.tensor.transpose(ptq[:], q_tm[:, k * P:(k + 1) * P], idf[:])
                        nc.vector.tensor_copy(q_fm[:, k, t * P:(t + 1) * P], ptq[:])
                        ptk = ps_tp.tile([P, P], F32, tag="tp")
                        nc.tensor.transpose(ptk[:], k_tm[:, k * P:(k + 1) * P], idf[:])
                        nc.vector.tensor_copy(k_fm[:, k, t * P:(t + 1) * P], ptk[:])

                _p2.__exit__(None, None, None)
                # --- phase 3: attention per (img, head) ---
                _p3 = sc_("p3attn"); _p3.__enter__()
                for img in range(2):
                    for hh in range(HEADS):
                        po = (hh % 2) * HD
                        ch = hh // 2
                        lt = ps_lt.tile([P, 2, NTOK], F32, tag="lt")
                        for mt in range(2):
                            nc.tensor.matmul(
                                lt[:, mt, :],
                                k_fm[po:po + HD, ch, img * NTOK + mt * P: img * NTOK + (mt + 1) * P],
                                q_fm[po:po + HD, ch, img * NTOK: (img + 1) * NTOK],
                                start=True, stop=True)
                        eT = eTp.tile([P, 2, NTOK], F32, tag="eT")
                        nc.scalar.activation(eT[:], lt[:], AF.Exp)
                        for nt in range(2):
                            oa = ps_oa.tile([P, HD + 1], F32, tag="oa")
                            for mt in range(2):
                                nc.tensor.matmul(
                                    oa[:], eT[:, mt, nt * P:(nt + 1) * P],
                                    v_aug[:, img * 2 + mt, hh, :],
                                    start=(mt == 0), stop=(mt == 1))
                            rinv = scp.tile([P, 1], F32, tag="sc")
                            nc.vector.reciprocal(rinv[:], oa[:, HD:HD + 1])
                            nc.scalar.activation(
                                o_tm[:, img * 2 + nt, hh * HD:(hh + 1) * HD],
                                oa[:, 0:HD], AF.Identity, scale=rinv[:])

                _p3.__exit__(None, None, None)
                # --- phase 4: proj + residual ---
                _p4 = sc_("p4proj"); _p4.__enter__()
                wp_tiles, wp_half = load_w(wproj_d, b, KD, DIM)
                cps = []
                for t in range(NT):
                    cps.append(quantize(o_tm[:, t, :], t, 4 * b + 1, KD, xqT))
                for t in range(NT):
                    for (cs, ce) in _mm_chunks(DIM):
                        pt = ps_mm.tile([P, 512], F32, tag="mm", name="pmm")[:, : ce - cs]
                        for k in range(KD):
                            wt = wp_tiles[k // wp_half]
                            nc.tensor.matmul(pt[:], xqT[:, k, t * P:(t + 1) * P],
                                             wt[:, k % wp_half, cs:ce],
                                             start=(k == 0), stop=(k == KD - 1))
                        tmp2 = tmp_.tile([P, DIM], F32, tag="tm", name="tmp2")[:, : ce - cs]
                        nc.scalar.activation(tmp2[:], pt[:], AF.Identity, scale=cps[t][:])
                        nc.vector.tensor_tensor(z[:, t, cs:ce], z[:, t, cs:ce], tmp2[:], OP.add)

                _p4.__exit__(None, None, None)
                # --- phase 5: norm2 + fc1 + gelu + quantize(g) ---
                _p5 = sc_("p5fc1"); _p5.__enter__()
                wf1_tiles, wf1_half = load_w(wfc1_d, b, KD, HID)
                c4s = []
                rstds2 = rstd_batch()
                for t in range(NT):
                    h = norm_mod(t, mt2, rstds2[t])
                    c3 = quantize(h[:], t, 4 * b + 2, KD, xqT)
                    g = gp.tile([P, HID], F32, tag="g")
                    for (cs, ce) in _mm_chunks(HID):
                        pt = ps_mm.tile([P, 512], F32, tag="mm", name="pmm")[:, : ce - cs]
                        for k in range(KD):
                            wt = wf1_tiles[k // wf1_half]
                            nc.tensor.matmul(pt[:], xqT[:, k, t * P:(t + 1) * P],
                                             wt[:, k % wf1_half, cs:ce],
                                             start=(k == 0), stop=(k == KD - 1))
                        nc.scalar.activation(g[:, cs:ce], pt[:], AF.Gelu_apprx_tanh,
                                             scale=c3[:])
                    c4s.append(quantize(g[:], t, 4 * b + 3, KH, xqT))

                _p5.__exit__(None, None, None)
                # --- phase 6: fc2 + residual ---
                _p6 = sc_("p6fc2"); _p6.__enter__()
                wf2_tiles, wf2_half = load_w(wfc2_d, b, KH, DIM)
                for t in range(NT):
                    for (cs, ce) in _mm_chunks(DIM):
                        pt = ps_mm.tile([P, 512], F32, tag="mm", name="pmm")[:, : ce - cs]
                        for k in range(KH):
                            wt = wf2_tiles[k // wf2_half]
                            nc.tensor.matmul(pt[:], xqT[:, k, t * P:(t + 1) * P],
                                             wt[:, k % wf2_half, cs:ce],
                                             start=(k == 0), stop=(k == KH - 1))
                        tmp2 = tmp_.tile([P, DIM], F32, tag="tm", name="tmp2")[:, : ce - cs]
                        nc.scalar.activation(tmp2[:], pt[:], AF.Identity, scale=c4s[t][:])
                        nc.vector.tensor_tensor(z[:, t, cs:ce], z[:, t, cs:ce], tmp2[:], OP.add)

                _p6.__exit__(None, None, None)
            # ---------------- final norm + head ----------------
            hw = wp.tile([P, KD, DIM], F32, tag="w")
            nc.sync.dma_start(hw[:], headWT_d.rearrange("(o p) d -> p o d", p=P))
            hbrow = rowp.tile([1, DIM], F32, tag="row")
            nc.sync.dma_start(hbrow[:], headb_d[:])
            hbb = constp.tile([P, DIM], F32)
            nc.gpsimd.partition_broadcast(hbb[:], hbrow[0:1, :])
            rstds_f = rstd_batch()
            for t in range(NT):
                zn = tmp_.tile([P, DIM], F32, tag="tm")
                nc.vector.tensor_scalar_mul(zn[:], z[:, t, :], rstds_f[t][:])
                znT = tmp_.tile([P, DIM], F32, tag="tm")
                for k in range(KD):
                    ptf = ps_tp.tile([P, P], F32, tag="tp")
                    nc.tensor.transpose(ptf[:], zn[:, k * P:(k + 1) * P], idf[:])
                    nc.vector.tensor_copy(znT[:, k * P:(k + 1) * P], ptf[:])
                for (cs, ce) in _mm_chunks(DIM):
                    pt = ps_mm.tile([P, 512], F32, tag="mm", name="pmm")[:, : ce - cs]
                    for k in range(KD):
                        nc.tensor.matmul(pt[:], znT[:, k * P:(k + 1) * P],
                                         hw[:, k, cs:ce], start=(k == 0), stop=(k == KD - 1))
                    ot = tmp_.tile([P, DIM], F32, tag="tm", name="ot")[:, : ce - cs]
                    nc.vector.tensor_tensor(ot[:], pt[:], hbb[:, cs:ce], OP.add)
                    nc.sync.dma_start(out_d[t * P:(t + 1) * P, cs:ce], ot[:])

    nc.compile()
    return nc


# ---------------------------------------------------------------------------
# host-side numerics (numpy, fp32 — matches jax CPU within ~1e-7)

def _gelu_tanh(x):
    x = x.astype(np.float32)
    c = np.float32(math.sqrt(2.0 / math.pi))
    return np.float32(0.5) * x * (np.float32(1.0) +
                                  np.tanh(c * (x + np.float32(0.044715) * x * x * x)))


def _time_embedding(t, t_w1, t_b1, t_w2, t_b2):
    half = DIM // 2
    freqs = np.exp(-np.log(10000.0) * np.arange(half, dtype=np.float32) / (half - 1)).astype(np.float32)
    args = t[:, None].astype(np.float32) * freqs[None, :]
    emb = np.concatenate([np.sin(args), np.cos(args)], axis=-1).astype(np.float32)
    h = _gelu_tanh(emb @ t_w1.T + t_b1)
    return (h @ t_w2.T + t_b2).astype(np.float32)


def _quant_w(w):
    ws = np.float32(np.mean(np.abs(w), dtype=np.float64)) + np.float32(1e-5)
    wq = np.clip(np.round(w.astype(np.float32) / ws), -1.0, 1.0)
    return wq, ws


def _prepare(inputs):
    x = np.asarray(inputs["x"], np.float32)
    t = np.asarray(inputs["t"], np.float32)
    B = x.shape[0]
    n_cores = 8
    per = B // n_cores  # 2
    p = PATCH
    hh = IMG // p

    xp = x.reshape(B, CIN, hh, p, hh, p).transpose(0, 2, 4, 1, 3, 5).reshape(B, hh * hh, CIN * p * p)

    t_emb = _time_embedding(t, inputs["t_w1"], inputs["t_b1"], inputs["t_w2"], inputs["t_b2"])
    silu = (t_emb / (1.0 + np.exp(-t_emb))).astype(np.float32)

    depth = DEPTH
    mods = np.zeros((depth, 2, B, 2, DIM), np.float32)  # [blk, norm, img, A/B, D]
    wscl = np.zeros((4 * depth,), np.float32)
    wq_all, wp_all, wf1_all, wf2_all = [], [], [], []
    for b in range(depth):
        mod = silu @ np.asarray(inputs["blk_ada_w"][b], np.float32).T + np.asarray(
            inputs["blk_ada_b"][b], np.float32)
        sh1, sc1, sh2, sc2 = np.split(mod, 4, axis=-1)
        n1 = np.asarray(inputs["blk_norm1"][b], np.float32)
        n2 = np.asarray(inputs["blk_norm2"][b], np.float32)
        mods[b, 0, :, 0, :] = n1[None, :] * (1.0 + sc1)
        mods[b, 0, :, 1, :] = sh1
        mods[b, 1, :, 0, :] = n2[None, :] * (1.0 + sc2)
        mods[b, 1, :, 1, :] = sh2

        for j, (nm, lst) in enumerate([("blk_qkv", wq_all), ("blk_proj", wp_all),
                                       ("blk_fc1", wf1_all), ("blk_fc2", wf2_all)]):
            wq, ws = _quant_w(np.asarray(inputs[nm][b], np.float32))
            lst.append(np.ascontiguousarray(wq.T).astype(ml_dtypes.bfloat16))
            wscl[4 * b + j] = ws / np.float32(127.0)

    wqkv = np.stack(wq_all)
    wproj = np.stack(wp_all)
    wfc1 = np.stack(wf1_all)
    wfc2 = np.stack(wf2_all)

    posb = (np.asarray(inputs["pos_embed"][0], np.float32) +
            np.asarray(inputs["patch_b"], np.float32)[None, :]).astype(np.float32)
    patchWT = np.ascontiguousarray(np.asarray(inputs["patch_w"], np.float32).T)
    norm_w = np.asarray(inputs["norm_w"], np.float32)
    headWT = np.ascontiguousarray(np.asarray(inputs["head_w"], np.float32).T * norm_w[:, None])
    headb = np.asarray(inputs["head_b"], np.float32)[None, :]

    key = ("prog", depth)
    if key not in _CACHED:
        _CACHED[key] = build_program(depth)
    nc = _CACHED[key]

    in_maps = []
    for c in range(n_cores):
        imgs = slice(c * per, (c + 1) * per)
        xpT = np.ascontiguousarray(xp[imgs].reshape(per * hh * hh, CIN * p * p).T)
        in_maps.append(dict(
            xpT=xpT, posb=posb, patchWT=patchWT, headWT=headWT, headb=headb,
            wqkv=wqkv, wproj=wproj, wfc1=wfc1, wfc2=wfc2,
            mods=np.ascontiguousarray(
                np.broadcast_to(mods[:, :, None, imgs], (depth, 2, 128, per, 2, DIM))),
            wscl=wscl[None, :],
        ))

    return nc, in_maps


def _assemble(res, B=16, per=2):
    p = PATCH
    hh = IMG // p
    out = np.zeros((B, CIN, IMG, IMG), np.float32)
    for c in range(B // per):
        zo = res.results[c]["zout"]  # [512, 768]
        for i in range(per):
            zi = zo[i * 256:(i + 1) * 256]
            out[c * per + i] = zi.reshape(hh, hh, CIN, p, p).transpose(2, 0, 3, 1, 4).reshape(CIN, IMG, IMG)
    return out


def kernel(**inputs):
    nc, in_maps = _prepare(inputs)
    res = run_bass_kernel_spmd(nc, in_maps, list(range(len(in_maps))), trace=False)
    return _assemble(res)
